# revision 21
# baseline (speedup 1.0000x reference)
"""DAHHConv (hypergraph conv) Trainium2 Bass kernel, 8-core SPMD.

Math (reference):
    x' = x @ theta                      # [B,N,C]  (folded on HOST)
    xe = (H^T x') / deg_e               # [B,E,C], deg_e = sum_n H
    xn = (H xe) / deg_n                 # [B,N,C], deg_n = sum_e H
    out = xn + bias                     # (bias on host)

Sharding: 8 cores = 4 batches x 2 e-halves; core c -> batch b=c//2,
half h=c%2. Both phases shard the HYPEREDGE dim: core (b,h) owns
e in [1024h, 1024h+1024).
  Phase 1 (edge aggregation, contract n): me[64,1024] = x'^T @ H_n
  over ALL N for the own e-half - fully local.
  Phase 3 (node aggregation, contract e): each core produces the
  PARTIAL y^T[64, 8192] = xe^T @ H_e^T over its own e-half for the
  FULL node range. The pair-sum over the two e-halves and the deg_n
  division happen in the host-side unshard (partial-sum gather), so the
  kernel needs NO inter-core collective (ncfw AllGather costs 40-60us
  wall, dwarfing the 133KB payload).

v3 structure (75.0us v1 baseline -> v2 74.4us -> here):
  - Every matmul is M=64 issued as tile_position (0,0)/(0,64) column
    pairs: measured 1.76x faster per moving byte than M=65 (250ns ->
    137ns per 512-row fp8 matmul). theta folded into x' on the host;
    1/deg_e supplied by the host (rd input); a host-built J=[I64;I64]
    stationary turns pair-sum + transpose into ONE small matmul per
    e-chunk.
  - The per-core HBM port caps at ~400-420 GB/s regardless of queue
    count (all queues share q_axi_port 0), so the kernel is DMA-bytes
    bound: 17.6MB -> ~44us floor. Every DMA issue costs ~0.6us of
    engine time per 128 descriptors, so H is host-packed PARTITION-
    MAJOR ([128, 64KB-contiguous-per-partition]) making multi-MB
    transfers cost 128 descriptors: the whole load plan is ~12 issues
    (v2: 25+), keeping the port saturated end-to-end.
  - Load order on the sync queue: hn pairs 0-1 (128KB, first matmul
    ~7us) -> hn ramp -> xp/jm/rd -> hn bulk -> ht bulk. Output stores
    ride the idle scalar queue; PSUM->SBUF casts split across vector
    and scalar engines.
"""

import numpy as np
import ml_dtypes

B, N, E, C = 4, 8192, 2048, 64
NCORES = 8
EH = E // 2          # 1024: e-range per core
NCHUNK = N // 128    # 64 n-chunks in phase 1
NPAIR = NCHUNK // 2  # 32 chunk pairs (stream A even, stream B odd)
ECHUNK = EH // 128   # 8 e-chunks in phase 3 (own half only)
NSPAN = 1024         # phase-3 output span (2 PSUM banks at fp32)
NSPANS = N // NSPAN  # 8 spans covering the FULL node range
BF16 = ml_dtypes.bfloat16
FP8 = ml_dtypes.float8_e4m3

_cache = {}


def _split_waits_json(raw: bytes) -> bytes:
    """BIR post-pass: this walrus/ISA build allows only ONE sync wait per
    instruction, but the Tile scheduler attaches several. Hoist all but
    the last wait of each instruction onto standalone EventSemaphore
    instructions inserted just before it on the same engine (waits are
    pure preconditions, so running them earlier on the same engine
    stream is equivalent)."""
    import json

    m = json.loads(raw)
    ctr = 0
    for f in m["functions"]:
        for blk in f["blocks"]:
            new = []
            for inst in blk["instructions"]:
                si = inst.get("sync_info")
                waits = (si or {}).get("on_wait") or []
                if len(waits) > 1:
                    for w in waits[:-1]:
                        ctr += 1
                        new.append(
                            {
                                "debug": inst.get("debug", 0),
                                "engine": inst["engine"],
                                "ins": [],
                                "name": f"{inst['name']}-xw{ctr}",
                                "opcode": "EventSemaphore",
                                "outs": [],
                                "sync_info": {"on_update": [], "on_wait": [w]},
                            }
                        )
                    si["on_wait"] = [waits[-1]]
                new.append(inst)
            blk["instructions"] = new
    return json.dumps(m).encode()


def build_bass():
    import concourse.bass as bass
    import concourse.mybir as mybir
    from concourse.tile import TileContext

    dt = mybir.dt
    nc = bass.Bass()

    # partition-major: hn[p, 1024j + e] = H[128j + p, e_own]; ht[p,
    # 8192s + 1024k + n'] = H[1024s + n', 128k + p]. 64KB contiguous
    # per partition -> 128 descriptors per DMA of any size.
    hn = nc.declare_dram_parameter("hn", [128, NCHUNK * EH], dt.float8e4,
                                   isOutput=False)
    ht = nc.declare_dram_parameter("ht", [128, NSPANS * ECHUNK * NSPAN],
                                   dt.float8e4, isOutput=False)
    xp = nc.declare_dram_parameter("xp", [128, NCHUNK * C], dt.bfloat16, isOutput=False)
    jm = nc.declare_dram_parameter("jm", [128, C], dt.bfloat16, isOutput=False)
    rd = nc.declare_dram_parameter("rd", [128, ECHUNK], dt.float32, isOutput=False)
    # PARTIAL y^T for the full node range. Host sums the pair and
    # divides by deg_n (partial-sum unshard).
    out = nc.declare_dram_parameter("out", [C, N], dt.bfloat16, isOutput=True)

    # hn DMA split points, in chunk units: fine at the head so the
    # first matmuls start early, then 1MB pieces -- coarser bulk makes
    # consumers cliff-wait on whole-DMA completion (dep granularity)
    # and the resulting PE gaps also drop it out of max p-state
    HN_SPLITS = [(0, 2), (2, 8), (8, 16), (16, 24), (24, 32), (32, 40),
                 (40, 48), (48, 56), (56, 64)]
    # ht DMA split: 1 span (1MB) each; span 7 splits into stream-A /
    # stream-B half-columns below so the last-arrival tail shrinks
    HT_SPLITS = [(0, 1), (1, 2), (2, 3), (3, 4), (4, 5), (5, 6), (6, 7)]

    with TileContext(nc) as tc:
        with (
            tc.tile_pool(name="const", bufs=1) as const,
            tc.tile_pool(name="persist", bufs=1) as persist,
            tc.tile_pool(name="opool", bufs=6) as opool,
        ):
            xp_sb = persist.tile([128, NCHUNK * C], dt.bfloat16)
            jm_sb = const.tile([128, C], dt.bfloat16)
            rd_sb = const.tile([128, ECHUNK], dt.float32)
            me_sb = persist.tile([128, EH], dt.bfloat16)
            xe_sb = persist.tile([128, ECHUNK * C], dt.bfloat16)
            hn_sb = persist.tile([128, NCHUNK * EH], dt.float8e4)
            ht_sb = persist.tile([128, NSPANS * ECHUNK * NSPAN], dt.float8e4)

            # ---- load plan (single sync HWDGE queue: FIFO = priority) ----
            # ALL stationaries + constants land before the hn bulk: a
            # late xp slice measurably stalled every pair behind it
            nc.sync.dma_start(hn_sb[:, 0 : EH * 2], hn[:, 0 : EH * 2])
            nc.sync.dma_start(xp_sb[:, 0:256], xp[:, 0:256])
            nc.sync.dma_start(xp_sb[:, 256:], xp[:, 256:])
            nc.sync.dma_start(jm_sb[:], jm[:])
            nc.sync.dma_start(rd_sb[:], rd[:])
            for lo, hi in HN_SPLITS[1:]:
                nc.sync.dma_start(hn_sb[:, EH * lo : EH * hi],
                                  hn[:, EH * lo : EH * hi])
            w = ECHUNK * NSPAN
            for lo, hi in HT_SPLITS:
                nc.sync.dma_start(ht_sb[:, w * lo : w * hi],
                                  ht[:, w * lo : w * hi])
            # span 7 as two half-column DMAs: stream A's matmuls run
            # while stream B's bytes are still in flight
            s7d = ht_sb[:, w * 7 : w * 8].rearrange("p (k n) -> p k n", n=NSPAN)
            s7s = ht[:, w * 7 : w * 8].rearrange("p (k n) -> p k n", n=NSPAN)
            nc.sync.dma_start(s7d[:, :, 0:512], s7s[:, :, 0:512])
            nc.sync.dma_start(s7d[:, :, 512:1024], s7s[:, :, 512:1024])

            # ---- phase 1: me[64,1024] = x'^T @ H_n, dual streams ----
            # stream A (quadrant col 0) takes even chunks -> ps_a parts
            # 0-63; stream B (col 64) odd chunks -> ps_b parts 64-127.
            # start=True clears a whole PSUM bank's has_written bits, so
            # each stream accumulates in its OWN banks.
            with tc.tile_pool(name="ps1", bufs=1, space="PSUM") as ps1:
                ps_a = ps1.tile([64, EH], dt.float32, tag="meA")
                ps_b = ps1.tile([128, EH], dt.float32, tag="meB")
                # A(h0),A(h1) then B(h0),B(h1): consecutive matmuls
                # share their stationary so codegen can skip the reload
                for m in range(NPAIR):
                    for half in range(2):
                        sl = 512 * half
                        nc.tensor.matmul(
                            ps_a[:, sl : sl + 512],
                            xp_sb[:, 128 * m : 128 * m + 64],
                            hn_sb[:, 2048 * m + sl : 2048 * m + sl + 512],
                            start=(m == 0),
                            stop=(m == NPAIR - 1),
                            tile_position=(0, 0),
                        )
                    for half in range(2):
                        sl = 512 * half
                        nc.tensor.matmul(
                            ps_b[64:128, sl : sl + 512],
                            xp_sb[:, 128 * m + 64 : 128 * m + 128],
                            hn_sb[:, 2048 * m + 1024 + sl : 2048 * m + 1024 + sl + 512],
                            start=(m == 0),
                            stop=(m == NPAIR - 1),
                            tile_position=(0, 64),
                        )

                # evict me streams to SBUF (bf16): A on parts 0-63 via
                # vector, B on 64-127 via scalar, each in halves so the
                # first J-matmuls start after ~0.35us
                nc.vector.tensor_copy(me_sb[0:64, 0:512], ps_a[:, 0:512])
                nc.scalar.copy(me_sb[64:128, 0:512], ps_b[64:128, 0:512])
                nc.vector.tensor_copy(me_sb[0:64, 512:1024], ps_a[:, 512:1024])
                nc.scalar.copy(me_sb[64:128, 512:1024], ps_b[64:128, 512:1024])

            # ---- phase 2: xe_k[128e,64c] = me_sb[:,k]^T @ [I;I] ----
            # one matmul per e-chunk does pair-sum + transpose; then a
            # per-partition scalar multiply applies 1/deg_e and casts
            with tc.tile_pool(name="psx", bufs=4, space="PSUM") as psx:
                for k in range(ECHUNK):
                    ps_x = psx.tile([128, C], dt.float32, tag="xe")
                    nc.tensor.matmul(
                        ps_x[:],
                        me_sb[:, 128 * k : 128 * (k + 1)],
                        jm_sb[:],
                        start=True,
                        stop=True,
                    )
                    nc.vector.tensor_scalar_mul(
                        xe_sb[:, C * k : C * (k + 1)], ps_x[:], rd_sb[:, k : k + 1]
                    )

            # ---- phase 3: partial y^T spans over the full N ----
            # dual streams again: A -> n-cols [0,512), B -> [512,1024).
            # Span-serial matches the 1MB-per-span supply stream (codegen
            # does NOT elide repeated LDWEIGHTS, so grouping spans only
            # couples each start to the later span's arrival).
            with tc.tile_pool(name="psY", bufs=3, space="PSUM") as psY:
                for s in range(NSPANS):
                    ps_ya = psY.tile([64, 512], dt.float32, tag="yA",
                                     name=f"yA{s}")
                    ps_yb = psY.tile([128, 512], dt.float32, tag="yB",
                                     name=f"yB{s}")
                    ho = 8192 * s
                    for k in range(ECHUNK):
                        nc.tensor.matmul(
                            ps_ya[:],
                            xe_sb[:, C * k : C * (k + 1)],
                            ht_sb[:, ho + NSPAN * k : ho + NSPAN * k + 512],
                            start=(k == 0),
                            stop=(k == ECHUNK - 1),
                            tile_position=(0, 0),
                        )
                        nc.tensor.matmul(
                            ps_yb[64:128, :],
                            xe_sb[:, C * k : C * (k + 1)],
                            ht_sb[:, ho + NSPAN * k + 512 : ho + NSPAN * (k + 1)],
                            start=(k == 0),
                            stop=(k == ECHUNK - 1),
                            tile_position=(0, 64),
                        )
                    # evictions all on vector (ACT copies measure 0.82us
                    # vs DVE 0.55us and scalar also issues the stores).
                    # Stores stay on scalar: a queue that mixes loads
                    # and stores loses completion-order guarantees and
                    # NaNs on cold runs (write acks pass read data).
                    o_sb = opool.tile([128, 512], dt.bfloat16, tag="o_sb")
                    nc.vector.tensor_copy(o_sb[0:64, :], ps_ya[:])
                    nc.vector.tensor_copy(o_sb[64:128, :], ps_yb[64:128, :])
                    nc.scalar.dma_start(
                        out[:, NSPAN * s : NSPAN * s + 512], o_sb[0:64, :]
                    )
                    nc.scalar.dma_start(
                        out[:, NSPAN * s + 512 : NSPAN * (s + 1)],
                        o_sb[64:128, :],
                    )

    orig_to_json = nc.to_json_bytes
    nc.to_json_bytes = lambda: _split_waits_json(orig_to_json())
    return nc


def _fp8_exact(a):
    # H is 0/1: 1.0 is exactly 0x38 in float8_e4m3.
    return (np.where(a != 0, 0x38, 0)).astype(np.uint8).view(FP8)


def _prepare_in_maps(x, H, theta):
    x = np.ascontiguousarray(x, dtype=np.float32)
    H = np.ascontiguousarray(H, dtype=np.float32)
    theta = np.asarray(theta, dtype=np.float32)
    _cache["rdeg_n"] = 1.0 / H.sum(axis=2)          # [B, N] for _assemble
    rdeg_e = 1.0 / H.sum(axis=1)                     # [B, E]
    jmat = np.concatenate([np.eye(C), np.eye(C)], axis=0).astype(BF16)
    in_maps = []
    for c in range(NCORES):
        b, h = divmod(c, 2)
        own = H[b, :, EH * h : EH * (h + 1)]            # [N, EH]
        # hn partition-major: [p, 1024j + e] = own[128j + p, e]
        hnc = _fp8_exact(np.ascontiguousarray(
            own.reshape(NCHUNK, 128, EH).transpose(1, 0, 2)
               .reshape(128, NCHUNK * EH)
        ))
        # ht partition-major: [p, 8192s + 1024k + n'] = own[1024s+n', 128k+p]
        t4 = own.reshape(NSPANS, NSPAN, ECHUNK, 128)
        htc = _fp8_exact(np.ascontiguousarray(
            t4.transpose(3, 0, 2, 1).reshape(128, NSPANS * ECHUNK * NSPAN)
        ))
        # x' = x @ theta packed to match hn: chunk j, partition p <->
        # node n = 128j + p; pair m = chunks (2m, 2m+1)
        xa = (x[b] @ theta).astype(BF16)                 # [N, 64]
        xpc = np.ascontiguousarray(
            xa.reshape(NCHUNK, 128, C).transpose(1, 0, 2)
              .reshape(128, NCHUNK * C)
        )
        # rd[p, k] = 1/deg_e[b, EH*h + 128k + p]
        rdc = np.ascontiguousarray(
            rdeg_e[b, EH * h : EH * (h + 1)].reshape(ECHUNK, 128).T
        ).astype(np.float32)
        in_maps.append({"hn": hnc, "ht": htc, "xp": xpc, "jm": jmat, "rd": rdc})
    return in_maps


def _assemble(results, bias):
    # partial-sum unshard: sum the pair's e-half contributions, divide
    # by deg_n (stashed by _prepare_in_maps), transpose, add bias
    rdeg = _cache["rdeg_n"]
    out = np.empty((B, N, C), dtype=np.float32)
    for b in range(B):
        r = (results[2 * b]["out"].astype(np.float32)
             + results[2 * b + 1]["out"].astype(np.float32))  # [C, N]
        out[b] = (r * rdeg[b][None, :]).T
    out += np.asarray(bias, dtype=np.float32)[None, None, :]
    return out


def get_nc():
    if "nc" not in _cache:
        _cache["nc"] = build_bass()
    return _cache["nc"]


def kernel(x, H, theta, bias):
    from concourse.bass_utils import run_bass_kernel_spmd

    nc = get_nc()
    in_maps = _prepare_in_maps(x, H, theta)
    res = run_bass_kernel_spmd(nc, in_maps, list(range(NCORES)))
    return _assemble(res.results, bias)


# revision 24
# speedup vs baseline: 1.0695x; 1.0695x over previous
"""DAHHConv (hypergraph conv) Trainium2 Bass kernel, 8-core SPMD.

Math (reference):
    x' = x @ theta                      # [B,N,C]  (folded on HOST)
    xe = (H^T x') / deg_e               # [B,E,C], deg_e = sum_n H
    xn = (H xe) / deg_n                 # [B,N,C], deg_n = sum_e H
    out = xn + bias                     # (bias on host)

Sharding: 8 cores = 4 batches x 2 e-halves; core c -> batch b=c//2,
half h=c%2. Both phases shard the HYPEREDGE dim: core (b,h) owns
e in [1024h, 1024h+1024).
  Phase 1 (edge aggregation, contract n): me[64,1024] = x'^T @ H_n
  over ALL N for the own e-half - fully local.
  Phase 3 (node aggregation, contract e): each core produces the
  PARTIAL y^T[64, 8192] = xe^T @ H_e^T over its own e-half for the
  FULL node range. The pair-sum over the two e-halves and the deg_n
  division happen in the host-side unshard (partial-sum gather), so the
  kernel needs NO inter-core collective (ncfw AllGather costs 40-60us
  wall, dwarfing the 133KB payload).

Final structure (v1 baseline 75.0us -> ~66-71us measured, port-bound):
  - Every matmul is M=64 issued as tile_position (0,0)/(0,64) column
    pairs; the two quadrant streams execute CONCURRENTLY on the PE
    (trace: A-matmul 0.18us, paired B-matmul 0.03us residual), putting
    both phases' compute floor (~13.4us) well under their DMA supply
    (~20us each). theta folded into x' on the host; 1/deg_e supplied
    by the host (rd input); a host-built J=[I64;I64] stationary turns
    pair-sum + transpose into ONE small matmul per e-chunk.
  - The per-core HBM port caps at ~420 GB/s regardless of queue count
    (all queues share q_axi_port 0), so the kernel is DMA-bytes bound:
    16.5MB of loads -> ~40us floor + ~7.5us NEFF preamble + ~5us
    boundary/eviction/drain tails. H must ride the port twice (both
    contraction layouts); fp8 keeps it exact (H is 0/1).
  - Every DMA issue costs ~0.6us of engine time per 128 descriptors,
    so H is host-packed PARTITION-MAJOR ([128, 64KB contiguous per
    partition]): multi-MB transfers cost 128 descriptors. Loads split
    ~1MB so consumers never cliff-wait on a whole transfer (which also
    drops the PE out of max p-state).
  - Loads live on the sync HWDGE queue ONLY, stores on scalar ONLY: a
    queue that mixes them loses completion-order guarantees and NaNs
    on cold runs (store write-acks pass load data). PSUM->SBUF casts
    on vector; boundary me-casts split vector/scalar.
"""

import numpy as np
import ml_dtypes

B, N, E, C = 4, 8192, 2048, 64
NCORES = 8
EH = E // 2          # 1024: e-range per core
NCHUNK = N // 128    # 64 n-chunks in phase 1
NPAIR = NCHUNK // 2  # 32 chunk pairs (stream A even, stream B odd)
ECHUNK = EH // 128   # 8 e-chunks in phase 3 (own half only)
NSPAN = 1024         # phase-3 output span (2 PSUM banks at fp32)
NSPANS = N // NSPAN  # 8 spans covering the FULL node range
BF16 = ml_dtypes.bfloat16
FP8 = ml_dtypes.float8_e4m3

_cache = {}


def _split_waits_json(raw: bytes) -> bytes:
    """BIR post-pass: this walrus/ISA build allows only ONE sync wait per
    instruction, but the Tile scheduler attaches several. Hoist all but
    the last wait of each instruction onto standalone EventSemaphore
    instructions inserted just before it on the same engine (waits are
    pure preconditions, so running them earlier on the same engine
    stream is equivalent)."""
    import json

    m = json.loads(raw)
    ctr = 0
    for f in m["functions"]:
        for blk in f["blocks"]:
            new = []
            for inst in blk["instructions"]:
                si = inst.get("sync_info")
                waits = (si or {}).get("on_wait") or []
                if len(waits) > 1:
                    for w in waits[:-1]:
                        ctr += 1
                        new.append(
                            {
                                "debug": inst.get("debug", 0),
                                "engine": inst["engine"],
                                "ins": [],
                                "name": f"{inst['name']}-xw{ctr}",
                                "opcode": "EventSemaphore",
                                "outs": [],
                                "sync_info": {"on_update": [], "on_wait": [w]},
                            }
                        )
                    si["on_wait"] = [waits[-1]]
                new.append(inst)
            blk["instructions"] = new
    return json.dumps(m).encode()


def build_bass():
    import concourse.bass as bass
    import concourse.mybir as mybir
    from concourse.tile import TileContext

    dt = mybir.dt
    nc = bass.Bass()

    # partition-major: hn[p, 1024j + e] = H[128j + p, e_own]; ht[p,
    # 8192s + 1024k + n'] = H[1024s + n', 128k + p]. 64KB contiguous
    # per partition -> 128 descriptors per DMA of any size.
    hn = nc.declare_dram_parameter("hn", [128, NCHUNK * EH], dt.float8e4,
                                   isOutput=False)
    ht = nc.declare_dram_parameter("ht", [128, NSPANS * ECHUNK * NSPAN],
                                   dt.float8e4, isOutput=False)
    xp = nc.declare_dram_parameter("xp", [128, NCHUNK * C], dt.bfloat16, isOutput=False)
    jm = nc.declare_dram_parameter("jm", [128, C], dt.bfloat16, isOutput=False)
    rd = nc.declare_dram_parameter("rd", [128, ECHUNK], dt.float32, isOutput=False)
    # PARTIAL y^T for the full node range. Host sums the pair and
    # divides by deg_n (partial-sum unshard).
    out = nc.declare_dram_parameter("out", [C, N], dt.bfloat16, isOutput=True)

    # hn DMA split points, in chunk units: fine at the head so the
    # first matmuls start early, then 1MB pieces -- coarser bulk makes
    # consumers cliff-wait on whole-DMA completion (dep granularity)
    # and the resulting PE gaps also drop it out of max p-state. The
    # tail is halved again so the last work chunk is small.
    HN_SPLITS = [(0, 2), (2, 8), (8, 16), (16, 24), (24, 32), (32, 40),
                 (40, 48), (48, 56), (56, 60), (60, 64)]
    # ht DMA split, in (span, chunk-lo, chunk-hi) units: 1 span (1MB)
    # each, with the last two spans halved by CHUNK RANGE (contiguous
    # 4KB runs -> full-rate 128-descriptor DMAs; a half-COLUMN split
    # was measured at ~100 GB/s from 512B descriptor runs)
    HT_SPLITS = [(0, 0, 8), (1, 0, 8), (2, 0, 8), (3, 0, 8), (4, 0, 8),
                 (5, 0, 8), (6, 0, 4), (6, 4, 8), (7, 0, 4), (7, 4, 8)]

    with TileContext(nc) as tc:
        with (
            tc.tile_pool(name="const", bufs=1) as const,
            tc.tile_pool(name="persist", bufs=1) as persist,
            tc.tile_pool(name="opool", bufs=6) as opool,
        ):
            xp_sb = persist.tile([128, NCHUNK * C], dt.bfloat16)
            jm_sb = const.tile([128, C], dt.bfloat16)
            rd_sb = const.tile([128, ECHUNK], dt.float32)
            me_sb = persist.tile([128, EH], dt.bfloat16)
            xe_sb = persist.tile([128, ECHUNK * C], dt.bfloat16)
            hn_sb = persist.tile([128, NCHUNK * EH], dt.float8e4)
            ht_sb = persist.tile([128, NSPANS * ECHUNK * NSPAN], dt.float8e4)

            # ---- load plan (single sync HWDGE queue: FIFO = priority) ----
            # ALL stationaries + constants land before the hn bulk: a
            # late xp slice measurably stalled every pair behind it
            nc.sync.dma_start(hn_sb[:, 0 : EH * 2], hn[:, 0 : EH * 2])
            nc.sync.dma_start(xp_sb[:, 0:256], xp[:, 0:256])
            nc.sync.dma_start(xp_sb[:, 256:], xp[:, 256:])
            nc.sync.dma_start(jm_sb[:], jm[:])
            nc.sync.dma_start(rd_sb[:], rd[:])
            for lo, hi in HN_SPLITS[1:]:
                nc.sync.dma_start(hn_sb[:, EH * lo : EH * hi],
                                  hn[:, EH * lo : EH * hi])
            w = ECHUNK * NSPAN
            for s, klo, khi in HT_SPLITS:
                lo = w * s + NSPAN * klo
                hi = w * s + NSPAN * khi
                nc.sync.dma_start(ht_sb[:, lo:hi], ht[:, lo:hi])

            # ---- phase 1: me[64,1024] = x'^T @ H_n, dual streams ----
            # stream A (quadrant col 0) takes even chunks -> ps_a parts
            # 0-63; stream B (col 64) odd chunks -> ps_b parts 64-127.
            # start=True clears a whole PSUM bank's has_written bits, so
            # each stream accumulates in its OWN banks.
            with tc.tile_pool(name="ps1", bufs=1, space="PSUM") as ps1:
                ps_a = ps1.tile([64, EH], dt.float32, tag="meA")
                ps_b = ps1.tile([128, EH], dt.float32, tag="meB")
                # A(h0),A(h1) then B(h0),B(h1): consecutive matmuls
                # share their stationary so codegen can skip the reload
                for m in range(NPAIR):
                    for half in range(2):
                        sl = 512 * half
                        nc.tensor.matmul(
                            ps_a[:, sl : sl + 512],
                            xp_sb[:, 128 * m : 128 * m + 64],
                            hn_sb[:, 2048 * m + sl : 2048 * m + sl + 512],
                            start=(m == 0),
                            stop=(m == NPAIR - 1),
                            tile_position=(0, 0),
                        )
                    for half in range(2):
                        sl = 512 * half
                        nc.tensor.matmul(
                            ps_b[64:128, sl : sl + 512],
                            xp_sb[:, 128 * m + 64 : 128 * m + 128],
                            hn_sb[:, 2048 * m + 1024 + sl : 2048 * m + 1024 + sl + 512],
                            start=(m == 0),
                            stop=(m == NPAIR - 1),
                            tile_position=(0, 64),
                        )

                # evict me streams to SBUF (bf16): A on parts 0-63 via
                # vector, B on 64-127 via scalar, each in halves so the
                # first J-matmuls start after ~0.35us
                nc.vector.tensor_copy(me_sb[0:64, 0:512], ps_a[:, 0:512])
                nc.scalar.copy(me_sb[64:128, 0:512], ps_b[64:128, 0:512])
                nc.vector.tensor_copy(me_sb[0:64, 512:1024], ps_a[:, 512:1024])
                nc.scalar.copy(me_sb[64:128, 512:1024], ps_b[64:128, 512:1024])

            # ---- phase 2: xe_k[128e,64c] = me_sb[:,k]^T @ [I;I] ----
            # one matmul per e-chunk does pair-sum + transpose; then a
            # per-partition scalar multiply applies 1/deg_e and casts
            with tc.tile_pool(name="psx", bufs=4, space="PSUM") as psx:
                for k in range(ECHUNK):
                    ps_x = psx.tile([128, C], dt.float32, tag="xe")
                    nc.tensor.matmul(
                        ps_x[:],
                        me_sb[:, 128 * k : 128 * (k + 1)],
                        jm_sb[:],
                        start=True,
                        stop=True,
                    )
                    nc.vector.tensor_scalar_mul(
                        xe_sb[:, C * k : C * (k + 1)], ps_x[:], rd_sb[:, k : k + 1]
                    )

            # ---- phase 3: partial y^T spans over the full N ----
            # dual streams again: A -> n-cols [0,512), B -> [512,1024).
            # Span-serial matches the 1MB-per-span supply stream (codegen
            # does NOT elide repeated LDWEIGHTS, so grouping spans only
            # couples each start to the later span's arrival).
            with tc.tile_pool(name="psY", bufs=3, space="PSUM") as psY:
                for s in range(NSPANS):
                    ps_ya = psY.tile([64, 512], dt.float32, tag="yA",
                                     name=f"yA{s}")
                    ps_yb = psY.tile([128, 512], dt.float32, tag="yB",
                                     name=f"yB{s}")
                    ho = 8192 * s
                    for k in range(ECHUNK):
                        nc.tensor.matmul(
                            ps_ya[:],
                            xe_sb[:, C * k : C * (k + 1)],
                            ht_sb[:, ho + NSPAN * k : ho + NSPAN * k + 512],
                            start=(k == 0),
                            stop=(k == ECHUNK - 1),
                            tile_position=(0, 0),
                        )
                        nc.tensor.matmul(
                            ps_yb[64:128, :],
                            xe_sb[:, C * k : C * (k + 1)],
                            ht_sb[:, ho + NSPAN * k + 512 : ho + NSPAN * (k + 1)],
                            start=(k == 0),
                            stop=(k == ECHUNK - 1),
                            tile_position=(0, 64),
                        )
                    # evictions all on vector (ACT copies measure 0.82us
                    # vs DVE 0.55us and scalar also issues the stores).
                    # Stores stay on scalar: a queue that mixes loads
                    # and stores loses completion-order guarantees and
                    # NaNs on cold runs (write acks pass read data).
                    o_sb = opool.tile([128, 512], dt.bfloat16, tag="o_sb")
                    nc.vector.tensor_copy(o_sb[0:64, :], ps_ya[:])
                    nc.vector.tensor_copy(o_sb[64:128, :], ps_yb[64:128, :])
                    nc.scalar.dma_start(
                        out[:, NSPAN * s : NSPAN * s + 512], o_sb[0:64, :]
                    )
                    nc.scalar.dma_start(
                        out[:, NSPAN * s + 512 : NSPAN * (s + 1)],
                        o_sb[64:128, :],
                    )

    orig_to_json = nc.to_json_bytes
    nc.to_json_bytes = lambda: _split_waits_json(orig_to_json())
    return nc


def _fp8_exact(a):
    # H is 0/1: 1.0 is exactly 0x38 in float8_e4m3.
    return (np.where(a != 0, 0x38, 0)).astype(np.uint8).view(FP8)


def _prepare_in_maps(x, H, theta):
    x = np.ascontiguousarray(x, dtype=np.float32)
    H = np.ascontiguousarray(H, dtype=np.float32)
    theta = np.asarray(theta, dtype=np.float32)
    _cache["rdeg_n"] = 1.0 / H.sum(axis=2)          # [B, N] for _assemble
    rdeg_e = 1.0 / H.sum(axis=1)                     # [B, E]
    jmat = np.concatenate([np.eye(C), np.eye(C)], axis=0).astype(BF16)
    in_maps = []
    for c in range(NCORES):
        b, h = divmod(c, 2)
        own = H[b, :, EH * h : EH * (h + 1)]            # [N, EH]
        # hn partition-major: [p, 1024j + e] = own[128j + p, e]
        hnc = _fp8_exact(np.ascontiguousarray(
            own.reshape(NCHUNK, 128, EH).transpose(1, 0, 2)
               .reshape(128, NCHUNK * EH)
        ))
        # ht partition-major: [p, 8192s + 1024k + n'] = own[1024s+n', 128k+p]
        t4 = own.reshape(NSPANS, NSPAN, ECHUNK, 128)
        htc = _fp8_exact(np.ascontiguousarray(
            t4.transpose(3, 0, 2, 1).reshape(128, NSPANS * ECHUNK * NSPAN)
        ))
        # x' = x @ theta packed to match hn: chunk j, partition p <->
        # node n = 128j + p; pair m = chunks (2m, 2m+1)
        xa = (x[b] @ theta).astype(BF16)                 # [N, 64]
        xpc = np.ascontiguousarray(
            xa.reshape(NCHUNK, 128, C).transpose(1, 0, 2)
              .reshape(128, NCHUNK * C)
        )
        # rd[p, k] = 1/deg_e[b, EH*h + 128k + p]
        rdc = np.ascontiguousarray(
            rdeg_e[b, EH * h : EH * (h + 1)].reshape(ECHUNK, 128).T
        ).astype(np.float32)
        in_maps.append({"hn": hnc, "ht": htc, "xp": xpc, "jm": jmat, "rd": rdc})
    return in_maps


def _assemble(results, bias):
    # partial-sum unshard: sum the pair's e-half contributions, divide
    # by deg_n (stashed by _prepare_in_maps), transpose, add bias
    rdeg = _cache["rdeg_n"]
    out = np.empty((B, N, C), dtype=np.float32)
    for b in range(B):
        r = (results[2 * b]["out"].astype(np.float32)
             + results[2 * b + 1]["out"].astype(np.float32))  # [C, N]
        out[b] = (r * rdeg[b][None, :]).T
    out += np.asarray(bias, dtype=np.float32)[None, None, :]
    return out


def get_nc():
    if "nc" not in _cache:
        _cache["nc"] = build_bass()
    return _cache["nc"]


def kernel(x, H, theta, bias):
    from concourse.bass_utils import run_bass_kernel_spmd

    nc = get_nc()
    in_maps = _prepare_in_maps(x, H, theta)
    res = run_bass_kernel_spmd(nc, in_maps, list(range(NCORES)))
    return _assemble(res.results, bias)


# revision 25
# speedup vs baseline: 1.1003x; 1.0288x over previous
"""DAHHConv (hypergraph conv) Trainium2 Bass kernel, 8-core SPMD.

Math (reference):
    x' = x @ theta                      # [B,N,C]  (folded on HOST)
    xe = (H^T x') / deg_e               # [B,E,C], deg_e = sum_n H
    xn = (H xe) / deg_n                 # [B,N,C], deg_n = sum_e H
    out = xn + bias                     # (bias on host)

Sharding: 8 cores = 4 batches x 2 e-halves; core c -> batch b=c//2,
half h=c%2. Both phases shard the HYPEREDGE dim: core (b,h) owns
e in [1024h, 1024h+1024).
  Phase 1 (edge aggregation, contract n): me[64,1024] = x'^T @ H_n
  over ALL N for the own e-half - fully local.
  Phase 3 (node aggregation, contract e): each core produces the
  PARTIAL y^T[64, 8192] = xe^T @ H_e^T over its own e-half for the
  FULL node range. The pair-sum over the two e-halves and the deg_n
  division happen in the host-side unshard (partial-sum gather), so the
  kernel needs NO inter-core collective (ncfw AllGather costs 40-60us
  wall, dwarfing the 133KB payload).

Final structure (v1 baseline 75.0us -> ~66-71us measured, port-bound):
  - Every matmul is M=64 issued as tile_position (0,0)/(0,64) column
    pairs; the two quadrant streams execute CONCURRENTLY on the PE
    (trace: A-matmul 0.18us, paired B-matmul 0.03us residual), putting
    both phases' compute floor (~13.4us) well under their DMA supply
    (~20us each). theta folded into x' on the host; 1/deg_e supplied
    by the host (rd input); a host-built J=[I64;I64] stationary turns
    pair-sum + transpose into ONE small matmul per e-chunk.
  - The per-core HBM port caps at ~420 GB/s regardless of queue count
    (all queues share q_axi_port 0), so the kernel is DMA-bytes bound:
    16.5MB of loads -> ~40us floor + ~7.5us NEFF preamble + ~5us
    boundary/eviction/drain tails. H must ride the port twice (both
    contraction layouts); fp8 keeps it exact (H is 0/1).
  - Every DMA issue costs ~0.6us of engine time per 128 descriptors,
    so H is host-packed PARTITION-MAJOR ([128, 64KB contiguous per
    partition]): multi-MB transfers cost 128 descriptors. Loads split
    ~1MB so consumers never cliff-wait on a whole transfer (which also
    drops the PE out of max p-state).
  - Loads live on the sync HWDGE queue ONLY, stores on scalar ONLY: a
    queue that mixes them loses completion-order guarantees and NaNs
    on cold runs (store write-acks pass load data). PSUM->SBUF casts
    on vector; boundary me-casts split vector/scalar.
"""

import numpy as np
import ml_dtypes

B, N, E, C = 4, 8192, 2048, 64
NCORES = 8
EH = E // 2          # 1024: e-range per core
NCHUNK = N // 128    # 64 n-chunks in phase 1
NPAIR = NCHUNK // 2  # 32 chunk pairs (stream A even, stream B odd)
ECHUNK = EH // 128   # 8 e-chunks in phase 3 (own half only)
NSPAN = 1024         # phase-3 output span (2 PSUM banks at fp32)
NSPANS = N // NSPAN  # 8 spans covering the FULL node range
BF16 = ml_dtypes.bfloat16
FP8 = ml_dtypes.float8_e4m3

_cache = {}


def _split_waits_json(raw: bytes) -> bytes:
    """BIR post-pass: this walrus/ISA build allows only ONE sync wait per
    instruction, but the Tile scheduler attaches several. Hoist all but
    the last wait of each instruction onto standalone EventSemaphore
    instructions inserted just before it on the same engine (waits are
    pure preconditions, so running them earlier on the same engine
    stream is equivalent)."""
    import json

    m = json.loads(raw)
    ctr = 0
    for f in m["functions"]:
        for blk in f["blocks"]:
            new = []
            for inst in blk["instructions"]:
                si = inst.get("sync_info")
                waits = (si or {}).get("on_wait") or []
                if len(waits) > 1:
                    for w in waits[:-1]:
                        ctr += 1
                        new.append(
                            {
                                "debug": inst.get("debug", 0),
                                "engine": inst["engine"],
                                "ins": [],
                                "name": f"{inst['name']}-xw{ctr}",
                                "opcode": "EventSemaphore",
                                "outs": [],
                                "sync_info": {"on_update": [], "on_wait": [w]},
                            }
                        )
                    si["on_wait"] = [waits[-1]]
                new.append(inst)
            blk["instructions"] = new
    return json.dumps(m).encode()


def build_bass():
    import concourse.bass as bass
    import concourse.mybir as mybir
    from concourse.tile import TileContext

    dt = mybir.dt
    nc = bass.Bass()

    # partition-major: hn[p, 1024j + e] = H[128j + p, e_own]; ht[p,
    # 8192s + 1024k + n'] = H[1024s + n', 128k + p]. 64KB contiguous
    # per partition -> 128 descriptors per DMA of any size.
    hn = nc.declare_dram_parameter("hn", [128, NCHUNK * EH], dt.float8e4,
                                   isOutput=False)
    ht = nc.declare_dram_parameter("ht", [128, NSPANS * ECHUNK * NSPAN],
                                   dt.float8e4, isOutput=False)
    xp = nc.declare_dram_parameter("xp", [128, NCHUNK * C], dt.bfloat16, isOutput=False)
    jm = nc.declare_dram_parameter("jm", [128, C], dt.bfloat16, isOutput=False)
    rd = nc.declare_dram_parameter("rd", [128, ECHUNK], dt.float32, isOutput=False)
    # PARTIAL y^T for the full node range. Host sums the pair and
    # divides by deg_n (partial-sum unshard).
    out = nc.declare_dram_parameter("out", [C, N], dt.bfloat16, isOutput=True)

    # hn DMA split points, in chunk units: fine at the head so the
    # first matmuls start early, then 1MB pieces -- coarser bulk makes
    # consumers cliff-wait on whole-DMA completion (dep granularity)
    # and the resulting PE gaps also drop it out of max p-state. The
    # tail is halved again so the last work chunk is small.
    HN_SPLITS = [(0, 2), (2, 8), (8, 16), (16, 24), (24, 32), (32, 40),
                 (40, 48), (48, 56), (56, 60), (60, 64)]
    # ht DMA split, in (span, chunk-lo, chunk-hi) units: 1 span (1MB)
    # each, with the last two spans halved by CHUNK RANGE (contiguous
    # 4KB runs -> full-rate 128-descriptor DMAs; a half-COLUMN split
    # was measured at ~100 GB/s from 512B descriptor runs)
    HT_SPLITS = [(0, 0, 8), (1, 0, 8), (2, 0, 8), (3, 0, 8), (4, 0, 8),
                 (5, 0, 8), (6, 0, 4), (6, 4, 8), (7, 0, 4), (7, 4, 8)]

    with TileContext(nc) as tc:
        with (
            tc.tile_pool(name="const", bufs=1) as const,
            tc.tile_pool(name="persist", bufs=1) as persist,
            tc.tile_pool(name="opool", bufs=6) as opool,
        ):
            xp_sb = persist.tile([128, NCHUNK * C], dt.bfloat16)
            jm_sb = const.tile([128, C], dt.bfloat16)
            rd_sb = const.tile([128, ECHUNK], dt.float32)
            me_sb = persist.tile([128, EH], dt.bfloat16)
            xe_sb = persist.tile([128, ECHUNK * C], dt.bfloat16)
            hn_sb = persist.tile([128, NCHUNK * EH], dt.float8e4)
            ht_sb = persist.tile([128, NSPANS * ECHUNK * NSPAN], dt.float8e4)

            # ---- load plan (single sync HWDGE queue: FIFO = priority) ----
            # ALL stationaries + constants land before the hn bulk: a
            # late xp slice measurably stalled every pair behind it
            nc.sync.dma_start(hn_sb[:, 0 : EH * 2], hn[:, 0 : EH * 2])
            nc.sync.dma_start(xp_sb[:, 0:256], xp[:, 0:256])
            nc.sync.dma_start(xp_sb[:, 256:], xp[:, 256:])
            nc.sync.dma_start(jm_sb[:], jm[:])
            nc.sync.dma_start(rd_sb[:], rd[:])
            for lo, hi in HN_SPLITS[1:]:
                nc.sync.dma_start(hn_sb[:, EH * lo : EH * hi],
                                  hn[:, EH * lo : EH * hi])
            w = ECHUNK * NSPAN
            for s, klo, khi in HT_SPLITS:
                lo = w * s + NSPAN * klo
                hi = w * s + NSPAN * khi
                nc.sync.dma_start(ht_sb[:, lo:hi], ht[:, lo:hi])

            # ---- phase 1: me[64,1024] = x'^T @ H_n, dual streams ----
            # stream A (quadrant col 0) takes even chunks -> ps_a parts
            # 0-63; stream B (col 64) odd chunks -> ps_b parts 64-127.
            # start=True clears a whole PSUM bank's has_written bits, so
            # each stream accumulates in its OWN banks.
            with tc.tile_pool(name="ps1", bufs=1, space="PSUM") as ps1:
                ps_a = ps1.tile([64, EH], dt.float32, tag="meA")
                ps_b = ps1.tile([128, EH], dt.float32, tag="meB")
                # A(h0),A(h1) then B(h0),B(h1) per pair; the (0,64)
                # quadrant stream executes concurrently with (0,0)'s
                for m in range(NPAIR):
                    for half in range(2):
                        sl = 512 * half
                        nc.tensor.matmul(
                            ps_a[:, sl : sl + 512],
                            xp_sb[:, 128 * m : 128 * m + 64],
                            hn_sb[:, 2048 * m + sl : 2048 * m + sl + 512],
                            start=(m == 0),
                            stop=(m == NPAIR - 1),
                            tile_position=(0, 0),
                        )
                    for half in range(2):
                        sl = 512 * half
                        nc.tensor.matmul(
                            ps_b[64:128, sl : sl + 512],
                            xp_sb[:, 128 * m + 64 : 128 * m + 128],
                            hn_sb[:, 2048 * m + 1024 + sl : 2048 * m + 1024 + sl + 512],
                            start=(m == 0),
                            stop=(m == NPAIR - 1),
                            tile_position=(0, 64),
                        )

                # evict me streams to SBUF (bf16): A on parts 0-63 via
                # vector, B on 64-127 via scalar, each in halves so the
                # first J-matmuls start after ~0.35us
                nc.vector.tensor_copy(me_sb[0:64, 0:512], ps_a[:, 0:512])
                nc.scalar.copy(me_sb[64:128, 0:512], ps_b[64:128, 0:512])
                nc.vector.tensor_copy(me_sb[0:64, 512:1024], ps_a[:, 512:1024])
                nc.scalar.copy(me_sb[64:128, 512:1024], ps_b[64:128, 512:1024])

            # ---- phase 2: xe_k[128e,64c] = me_sb[:,k]^T @ [I;I] ----
            # one matmul per e-chunk does pair-sum + transpose; then a
            # per-partition scalar multiply applies 1/deg_e and casts
            with tc.tile_pool(name="psx", bufs=4, space="PSUM") as psx:
                for k in range(ECHUNK):
                    ps_x = psx.tile([128, C], dt.float32, tag="xe")
                    nc.tensor.matmul(
                        ps_x[:],
                        me_sb[:, 128 * k : 128 * (k + 1)],
                        jm_sb[:],
                        start=True,
                        stop=True,
                    )
                    nc.vector.tensor_scalar_mul(
                        xe_sb[:, C * k : C * (k + 1)], ps_x[:], rd_sb[:, k : k + 1]
                    )

            # ---- phase 3: partial y^T spans over the full N ----
            # dual streams again: A -> n-cols [0,512), B -> [512,1024).
            # Span-serial matches the 1MB-per-span supply stream (codegen
            # does NOT elide repeated LDWEIGHTS, so grouping spans only
            # couples each start to the later span's arrival).
            with tc.tile_pool(name="psY", bufs=3, space="PSUM") as psY:
                for s in range(NSPANS):
                    ps_ya = psY.tile([64, 512], dt.float32, tag="yA",
                                     name=f"yA{s}")
                    ps_yb = psY.tile([128, 512], dt.float32, tag="yB",
                                     name=f"yB{s}")
                    ho = 8192 * s
                    for k in range(ECHUNK):
                        nc.tensor.matmul(
                            ps_ya[:],
                            xe_sb[:, C * k : C * (k + 1)],
                            ht_sb[:, ho + NSPAN * k : ho + NSPAN * k + 512],
                            start=(k == 0),
                            stop=(k == ECHUNK - 1),
                            tile_position=(0, 0),
                        )
                        nc.tensor.matmul(
                            ps_yb[64:128, :],
                            xe_sb[:, C * k : C * (k + 1)],
                            ht_sb[:, ho + NSPAN * k + 512 : ho + NSPAN * (k + 1)],
                            start=(k == 0),
                            stop=(k == ECHUNK - 1),
                            tile_position=(0, 64),
                        )
                    # evictions all on vector (ACT copies measure 0.82us
                    # vs DVE 0.55us and scalar also issues the stores).
                    # Stores stay on scalar: a queue that mixes loads
                    # and stores loses completion-order guarantees and
                    # NaNs on cold runs (write acks pass read data).
                    o_sb = opool.tile([128, 512], dt.bfloat16, tag="o_sb")
                    nc.vector.tensor_copy(o_sb[0:64, :], ps_ya[:])
                    nc.vector.tensor_copy(o_sb[64:128, :], ps_yb[64:128, :])
                    nc.scalar.dma_start(
                        out[:, NSPAN * s : NSPAN * s + 512], o_sb[0:64, :]
                    )
                    nc.scalar.dma_start(
                        out[:, NSPAN * s + 512 : NSPAN * (s + 1)],
                        o_sb[64:128, :],
                    )

    orig_to_json = nc.to_json_bytes
    nc.to_json_bytes = lambda: _split_waits_json(orig_to_json())
    return nc


def _fp8_exact(a):
    # H is 0/1: 1.0 is exactly 0x38 in float8_e4m3.
    return (np.where(a != 0, 0x38, 0)).astype(np.uint8).view(FP8)


def _prepare_in_maps(x, H, theta):
    x = np.ascontiguousarray(x, dtype=np.float32)
    H = np.ascontiguousarray(H, dtype=np.float32)
    theta = np.asarray(theta, dtype=np.float32)
    _cache["rdeg_n"] = 1.0 / H.sum(axis=2)          # [B, N] for _assemble
    rdeg_e = 1.0 / H.sum(axis=1)                     # [B, E]
    jmat = np.concatenate([np.eye(C), np.eye(C)], axis=0).astype(BF16)
    in_maps = []
    for c in range(NCORES):
        b, h = divmod(c, 2)
        own = H[b, :, EH * h : EH * (h + 1)]            # [N, EH]
        # hn partition-major: [p, 1024j + e] = own[128j + p, e]
        hnc = _fp8_exact(np.ascontiguousarray(
            own.reshape(NCHUNK, 128, EH).transpose(1, 0, 2)
               .reshape(128, NCHUNK * EH)
        ))
        # ht partition-major: [p, 8192s + 1024k + n'] = own[1024s+n', 128k+p]
        t4 = own.reshape(NSPANS, NSPAN, ECHUNK, 128)
        htc = _fp8_exact(np.ascontiguousarray(
            t4.transpose(3, 0, 2, 1).reshape(128, NSPANS * ECHUNK * NSPAN)
        ))
        # x' = x @ theta packed to match hn: chunk j, partition p <->
        # node n = 128j + p; pair m = chunks (2m, 2m+1)
        xa = (x[b] @ theta).astype(BF16)                 # [N, 64]
        xpc = np.ascontiguousarray(
            xa.reshape(NCHUNK, 128, C).transpose(1, 0, 2)
              .reshape(128, NCHUNK * C)
        )
        # rd[p, k] = 1/deg_e[b, EH*h + 128k + p]
        rdc = np.ascontiguousarray(
            rdeg_e[b, EH * h : EH * (h + 1)].reshape(ECHUNK, 128).T
        ).astype(np.float32)
        in_maps.append({"hn": hnc, "ht": htc, "xp": xpc, "jm": jmat, "rd": rdc})
    return in_maps


def _assemble(results, bias):
    # partial-sum unshard: sum the pair's e-half contributions, divide
    # by deg_n (stashed by _prepare_in_maps), transpose, add bias
    rdeg = _cache["rdeg_n"]
    out = np.empty((B, N, C), dtype=np.float32)
    for b in range(B):
        r = (results[2 * b]["out"].astype(np.float32)
             + results[2 * b + 1]["out"].astype(np.float32))  # [C, N]
        out[b] = (r * rdeg[b][None, :]).T
    out += np.asarray(bias, dtype=np.float32)[None, None, :]
    return out


def get_nc():
    if "nc" not in _cache:
        _cache["nc"] = build_bass()
    return _cache["nc"]


def kernel(x, H, theta, bias):
    from concourse.bass_utils import run_bass_kernel_spmd

    nc = get_nc()
    in_maps = _prepare_in_maps(x, H, theta)
    res = run_bass_kernel_spmd(nc, in_maps, list(range(NCORES)))
    return _assemble(res.results, bias)


# revision 27
# speedup vs baseline: 1.1086x; 1.0075x over previous
"""DAHHConv (hypergraph conv) Trainium2 Bass kernel, 8-core SPMD.

Math (reference):
    x' = x @ theta                      # [B,N,C]  (folded on HOST)
    xe = (H^T x') / deg_e               # [B,E,C], deg_e = sum_n H
    xn = (H xe) / deg_n                 # [B,N,C], deg_n = sum_e H
    out = xn + bias                     # (bias on host)

Sharding: 8 cores = 4 batches x 2 e-halves; core c -> batch b=c//2,
half h=c%2. Both phases shard the HYPEREDGE dim: core (b,h) owns
e in [1024h, 1024h+1024).
  Phase 1 (edge aggregation, contract n): me[64,1024] = x'^T @ H_n
  over ALL N for the own e-half - fully local.
  Phase 3 (node aggregation, contract e): each core produces the
  PARTIAL y^T[64, 8192] = xe^T @ H_e^T over its own e-half for the
  FULL node range. The pair-sum over the two e-halves and the deg_n
  division happen in the host-side unshard (partial-sum gather), so the
  kernel needs NO inter-core collective (ncfw AllGather costs 40-60us
  wall, dwarfing the 133KB payload).

Final structure (v1 baseline 75.0us -> ~66-71us measured, port-bound):
  - Every matmul is M=64 issued as tile_position (0,0)/(0,64) column
    pairs; the two quadrant streams execute CONCURRENTLY on the PE
    (trace: A-matmul 0.18us, paired B-matmul 0.03us residual), putting
    both phases' compute floor (~13.4us) well under their DMA supply
    (~20us each). theta folded into x' on the host; 1/deg_e supplied
    by the host (rd input); a host-built J=[I64;I64] stationary turns
    pair-sum + transpose into ONE small matmul per e-chunk.
  - The per-core HBM port caps at ~420 GB/s regardless of queue count
    (all queues share q_axi_port 0), so the kernel is DMA-bytes bound:
    16.5MB of loads -> ~40us floor + ~7.5us NEFF preamble + ~5us
    boundary/eviction/drain tails. H must ride the port twice (both
    contraction layouts); fp8 keeps it exact (H is 0/1).
  - Every DMA issue costs ~0.6us of engine time per 128 descriptors,
    so H is host-packed PARTITION-MAJOR ([128, 64KB contiguous per
    partition]): multi-MB transfers cost 128 descriptors. Loads split
    ~1MB so consumers never cliff-wait on a whole transfer (which also
    drops the PE out of max p-state).
  - Loads live on the sync HWDGE queue ONLY, stores on scalar ONLY: a
    queue that mixes them loses completion-order guarantees and NaNs
    on cold runs (store write-acks pass load data). PSUM->SBUF casts
    on vector; boundary me-casts split vector/scalar.
"""

import numpy as np
import ml_dtypes

B, N, E, C = 4, 8192, 2048, 64
NCORES = 8
EH = E // 2          # 1024: e-range per core
NCHUNK = N // 128    # 64 n-chunks in phase 1
NPAIR = NCHUNK // 2  # 32 chunk pairs (stream A even, stream B odd)
ECHUNK = EH // 128   # 8 e-chunks in phase 3 (own half only)
NSPAN = 1024         # phase-3 output span (2 PSUM banks at fp32)
NSPANS = N // NSPAN  # 8 spans covering the FULL node range
BF16 = ml_dtypes.bfloat16
FP8 = ml_dtypes.float8_e4m3

_cache = {}


def _split_waits_json(raw: bytes) -> bytes:
    """BIR post-pass: this walrus/ISA build allows only ONE sync wait per
    instruction, but the Tile scheduler attaches several. Hoist all but
    the last wait of each instruction onto standalone EventSemaphore
    instructions inserted just before it on the same engine (waits are
    pure preconditions, so running them earlier on the same engine
    stream is equivalent)."""
    import json

    m = json.loads(raw)
    ctr = 0
    for f in m["functions"]:
        for blk in f["blocks"]:
            new = []
            for inst in blk["instructions"]:
                si = inst.get("sync_info")
                waits = (si or {}).get("on_wait") or []
                if len(waits) > 1:
                    for w in waits[:-1]:
                        ctr += 1
                        new.append(
                            {
                                "debug": inst.get("debug", 0),
                                "engine": inst["engine"],
                                "ins": [],
                                "name": f"{inst['name']}-xw{ctr}",
                                "opcode": "EventSemaphore",
                                "outs": [],
                                "sync_info": {"on_update": [], "on_wait": [w]},
                            }
                        )
                    si["on_wait"] = [waits[-1]]
                new.append(inst)
            blk["instructions"] = new
    return json.dumps(m).encode()


def build_bass():
    import concourse.bass as bass
    import concourse.mybir as mybir
    from concourse.tile import TileContext

    dt = mybir.dt
    nc = bass.Bass()

    # partition-major: hn[p, 1024j + e] = H[128j + p, e_own]; ht[p,
    # 8192s + 1024k + n'] = H[1024s + n', 128k + p]. 64KB contiguous
    # per partition -> 128 descriptors per DMA of any size.
    hn = nc.declare_dram_parameter("hn", [128, NCHUNK * EH], dt.float8e4,
                                   isOutput=False)
    ht = nc.declare_dram_parameter("ht", [128, NSPANS * ECHUNK * NSPAN],
                                   dt.float8e4, isOutput=False)
    xp = nc.declare_dram_parameter("xp", [128, NCHUNK * C], dt.bfloat16, isOutput=False)
    jm = nc.declare_dram_parameter("jm", [128, C], dt.bfloat16, isOutput=False)
    rd = nc.declare_dram_parameter("rd", [128, ECHUNK], dt.float32, isOutput=False)
    # PARTIAL y^T for the full node range. Host sums the pair and
    # divides by deg_n (partial-sum unshard).
    out = nc.declare_dram_parameter("out", [C, N], dt.bfloat16, isOutput=True)

    # hn DMA split points, in chunk units: fine at the head so the
    # first matmuls start early, then 1MB pieces -- coarser bulk makes
    # consumers cliff-wait on whole-DMA completion (dep granularity)
    # and the resulting PE gaps also drop it out of max p-state. The
    # tail is halved again so the last work chunk is small.
    HN_SPLITS = [(0, 2), (2, 8), (8, 16), (16, 24), (24, 32), (32, 40),
                 (40, 48), (48, 56), (56, 60), (60, 64)]
    # ht DMA split, in (span, chunk-lo, chunk-hi) units: 1 span (1MB)
    # each, with the last two spans halved by CHUNK RANGE (contiguous
    # 4KB runs -> full-rate 128-descriptor DMAs; a half-COLUMN split
    # was measured at ~100 GB/s from 512B descriptor runs)
    HT_SPLITS = [(0, 0, 8), (1, 0, 8), (2, 0, 8), (3, 0, 8), (4, 0, 8),
                 (5, 0, 8), (6, 0, 4), (6, 4, 8), (7, 0, 4), (7, 4, 8)]

    with TileContext(nc) as tc:
        with (
            tc.tile_pool(name="persist", bufs=1) as persist,
            tc.tile_pool(name="opool", bufs=6) as opool,
        ):
            xp_sb = persist.tile([128, NCHUNK * C], dt.bfloat16)
            jm_sb = persist.tile([128, C], dt.bfloat16)
            rd_sb = persist.tile([128, ECHUNK], dt.float32)
            me_sb = persist.tile([128, EH], dt.bfloat16)
            xe_sb = persist.tile([128, ECHUNK * C], dt.bfloat16)
            hn_sb = persist.tile([128, NCHUNK * EH], dt.float8e4)
            ht_sb = persist.tile([128, NSPANS * ECHUNK * NSPAN], dt.float8e4)

            # ---- load plan (single sync HWDGE queue: FIFO = priority) ----
            # ALL stationaries + constants land before the hn bulk: a
            # late xp slice measurably stalled every pair behind it
            nc.sync.dma_start(hn_sb[:, 0 : EH * 2], hn[:, 0 : EH * 2])
            nc.sync.dma_start(xp_sb[:, 0:256], xp[:, 0:256])
            nc.sync.dma_start(xp_sb[:, 256:], xp[:, 256:])
            nc.sync.dma_start(jm_sb[:], jm[:])
            nc.sync.dma_start(rd_sb[:], rd[:])
            for lo, hi in HN_SPLITS[1:]:
                nc.sync.dma_start(hn_sb[:, EH * lo : EH * hi],
                                  hn[:, EH * lo : EH * hi])
            w = ECHUNK * NSPAN
            for s, klo, khi in HT_SPLITS:
                lo = w * s + NSPAN * klo
                hi = w * s + NSPAN * khi
                nc.sync.dma_start(ht_sb[:, lo:hi], ht[:, lo:hi])

            # ---- phase 1: me[64,1024] = x'^T @ H_n, dual streams ----
            # stream A (quadrant col 0) takes even chunks -> ps_a parts
            # 0-63; stream B (col 64) odd chunks -> ps_b parts 64-127.
            # start=True clears a whole PSUM bank's has_written bits, so
            # each stream accumulates in its OWN banks.
            with tc.tile_pool(name="ps1", bufs=1, space="PSUM") as ps1:
                ps_a = ps1.tile([64, EH], dt.float32, tag="meA")
                ps_b = ps1.tile([128, EH], dt.float32, tag="meB")
                # A(h0),A(h1) then B(h0),B(h1) per pair; the (0,64)
                # quadrant stream executes concurrently with (0,0)'s
                for m in range(NPAIR):
                    for half in range(2):
                        sl = 512 * half
                        nc.tensor.matmul(
                            ps_a[:, sl : sl + 512],
                            xp_sb[:, 128 * m : 128 * m + 64],
                            hn_sb[:, 2048 * m + sl : 2048 * m + sl + 512],
                            start=(m == 0),
                            stop=(m == NPAIR - 1),
                            tile_position=(0, 0),
                        )
                    for half in range(2):
                        sl = 512 * half
                        nc.tensor.matmul(
                            ps_b[64:128, sl : sl + 512],
                            xp_sb[:, 128 * m + 64 : 128 * m + 128],
                            hn_sb[:, 2048 * m + 1024 + sl : 2048 * m + 1024 + sl + 512],
                            start=(m == 0),
                            stop=(m == NPAIR - 1),
                            tile_position=(0, 64),
                        )

                # evict me streams to SBUF (bf16): A on parts 0-63 via
                # vector, B on 64-127 via scalar, each in halves so the
                # first J-matmuls start after ~0.35us
                nc.vector.tensor_copy(me_sb[0:64, 0:512], ps_a[:, 0:512])
                nc.scalar.copy(me_sb[64:128, 0:512], ps_b[64:128, 0:512])
                nc.vector.tensor_copy(me_sb[0:64, 512:1024], ps_a[:, 512:1024])
                nc.scalar.copy(me_sb[64:128, 512:1024], ps_b[64:128, 512:1024])

            # ---- phase 2: xe_k[128e,64c] = me_sb[:,k]^T @ [I;I] ----
            # one matmul per e-chunk does pair-sum + transpose; then a
            # per-partition scalar multiply applies 1/deg_e and casts
            with tc.tile_pool(name="psx", bufs=4, space="PSUM") as psx:
                for k in range(ECHUNK):
                    ps_x = psx.tile([128, C], dt.float32, tag="xe")
                    nc.tensor.matmul(
                        ps_x[:],
                        me_sb[:, 128 * k : 128 * (k + 1)],
                        jm_sb[:],
                        start=True,
                        stop=True,
                    )
                    nc.vector.tensor_scalar_mul(
                        xe_sb[:, C * k : C * (k + 1)], ps_x[:], rd_sb[:, k : k + 1]
                    )

            # ---- phase 3: partial y^T spans over the full N ----
            # dual streams again: A -> n-cols [0,512), B -> [512,1024).
            # Span-serial matches the 1MB-per-span supply stream (codegen
            # does NOT elide repeated LDWEIGHTS, so grouping spans only
            # couples each start to the later span's arrival).
            with tc.tile_pool(name="psY", bufs=3, space="PSUM") as psY:
                for s in range(NSPANS):
                    ps_ya = psY.tile([64, 512], dt.float32, tag="yA",
                                     name=f"yA{s}")
                    ps_yb = psY.tile([128, 512], dt.float32, tag="yB",
                                     name=f"yB{s}")
                    ho = 8192 * s
                    for k in range(ECHUNK):
                        nc.tensor.matmul(
                            ps_ya[:],
                            xe_sb[:, C * k : C * (k + 1)],
                            ht_sb[:, ho + NSPAN * k : ho + NSPAN * k + 512],
                            start=(k == 0),
                            stop=(k == ECHUNK - 1),
                            tile_position=(0, 0),
                        )
                        nc.tensor.matmul(
                            ps_yb[64:128, :],
                            xe_sb[:, C * k : C * (k + 1)],
                            ht_sb[:, ho + NSPAN * k + 512 : ho + NSPAN * (k + 1)],
                            start=(k == 0),
                            stop=(k == ECHUNK - 1),
                            tile_position=(0, 64),
                        )
                    # evictions all on vector (ACT copies measure 0.82us
                    # vs DVE 0.55us and scalar also issues the stores).
                    # Stores stay on scalar: a queue that mixes loads
                    # and stores loses completion-order guarantees and
                    # NaNs on cold runs (write acks pass read data).
                    o_sb = opool.tile([128, 512], dt.bfloat16, tag="o_sb")
                    nc.vector.tensor_copy(o_sb[0:64, :], ps_ya[:])
                    if s == NSPANS - 1:
                        # final span: evict both halves in parallel
                        # (vector + scalar) to shorten the exit tail
                        nc.scalar.copy(o_sb[64:128, :], ps_yb[64:128, :])
                    else:
                        nc.vector.tensor_copy(o_sb[64:128, :], ps_yb[64:128, :])
                    nc.scalar.dma_start(
                        out[:, NSPAN * s : NSPAN * s + 512], o_sb[0:64, :]
                    )
                    nc.scalar.dma_start(
                        out[:, NSPAN * s + 512 : NSPAN * (s + 1)],
                        o_sb[64:128, :],
                    )

    orig_to_json = nc.to_json_bytes
    nc.to_json_bytes = lambda: _split_waits_json(orig_to_json())
    return nc


def _fp8_exact(a):
    # H is 0/1: 1.0 is exactly 0x38 in float8_e4m3.
    return (np.where(a != 0, 0x38, 0)).astype(np.uint8).view(FP8)


def _prepare_in_maps(x, H, theta):
    x = np.ascontiguousarray(x, dtype=np.float32)
    H = np.ascontiguousarray(H, dtype=np.float32)
    theta = np.asarray(theta, dtype=np.float32)
    _cache["rdeg_n"] = 1.0 / H.sum(axis=2)          # [B, N] for _assemble
    rdeg_e = 1.0 / H.sum(axis=1)                     # [B, E]
    jmat = np.concatenate([np.eye(C), np.eye(C)], axis=0).astype(BF16)
    in_maps = []
    for c in range(NCORES):
        b, h = divmod(c, 2)
        own = H[b, :, EH * h : EH * (h + 1)]            # [N, EH]
        # hn partition-major: [p, 1024j + e] = own[128j + p, e]
        hnc = _fp8_exact(np.ascontiguousarray(
            own.reshape(NCHUNK, 128, EH).transpose(1, 0, 2)
               .reshape(128, NCHUNK * EH)
        ))
        # ht partition-major: [p, 8192s + 1024k + n'] = own[1024s+n', 128k+p]
        t4 = own.reshape(NSPANS, NSPAN, ECHUNK, 128)
        htc = _fp8_exact(np.ascontiguousarray(
            t4.transpose(3, 0, 2, 1).reshape(128, NSPANS * ECHUNK * NSPAN)
        ))
        # x' = x @ theta packed to match hn: chunk j, partition p <->
        # node n = 128j + p; pair m = chunks (2m, 2m+1)
        xa = (x[b] @ theta).astype(BF16)                 # [N, 64]
        xpc = np.ascontiguousarray(
            xa.reshape(NCHUNK, 128, C).transpose(1, 0, 2)
              .reshape(128, NCHUNK * C)
        )
        # rd[p, k] = 1/deg_e[b, EH*h + 128k + p]
        rdc = np.ascontiguousarray(
            rdeg_e[b, EH * h : EH * (h + 1)].reshape(ECHUNK, 128).T
        ).astype(np.float32)
        in_maps.append({"hn": hnc, "ht": htc, "xp": xpc, "jm": jmat, "rd": rdc})
    return in_maps


def _assemble(results, bias):
    # partial-sum unshard: sum the pair's e-half contributions, divide
    # by deg_n (stashed by _prepare_in_maps), transpose, add bias
    rdeg = _cache["rdeg_n"]
    out = np.empty((B, N, C), dtype=np.float32)
    for b in range(B):
        r = (results[2 * b]["out"].astype(np.float32)
             + results[2 * b + 1]["out"].astype(np.float32))  # [C, N]
        out[b] = (r * rdeg[b][None, :]).T
    out += np.asarray(bias, dtype=np.float32)[None, None, :]
    return out


def get_nc():
    if "nc" not in _cache:
        _cache["nc"] = build_bass()
    return _cache["nc"]


def kernel(x, H, theta, bias):
    from concourse.bass_utils import run_bass_kernel_spmd

    nc = get_nc()
    in_maps = _prepare_in_maps(x, H, theta)
    res = run_bass_kernel_spmd(nc, in_maps, list(range(NCORES)))
    return _assemble(res.results, bias)
